# revision 28
# baseline (speedup 1.0000x reference)
"""CARAFE kernel for Trainium2 (8 NeuronCores, batch-parallel), v4.

Reference computation per image:
  R = relu(conv1x1(x, w_compress, b_compress))          [48, 128, 128]
  E = conv3x3(R, w_encoder, b_encoder, pad=1)           [100, 128, 128]
  Y = softmax over k of E.reshape(4, 25, H, W)          (s, k, h, w)
  out[s,c,h,w] = sum_k Y[s,k,h,w] * xpad[c, h+dy, w+dx] (k=(dy,dx), 5x5, pad 2)
  pixel-shuffle: out_ref[s*16 + c//4, 2h + (c//2)%2, 2w + c%2] = out[s,c,h,w]

v4 design (vs v2/v3):
  - The 25-tap patch sum is elementwise (per-pixel weights), but only the
    PRODUCTS need an elementwise engine.  The k-SUM moves to the Tensor
    engine: DVE writes each product plane P_k = Y_k * x_k to SBUF, and PE
    accumulates all 25 planes into PSUM via identity matmuls
    (psum[h,n] += sum_p I[p,h] P_k[p,n]).  This halves DVE work - DVE does
    25 muls per (s, chunk) instead of 25 muls + 24 adds - and the
    accumulation becomes exact fp32.
  - gpsimd does NO elementwise work: measured on HW, concurrent Pool
    activity slows DVE ops ~4x (global SBUF port contention), making any
    gpsimd offload net-negative.
  - Whole dataflow is pipelined in 32-column chunks (conv1x1 -> conv3x3 ->
    softmax -> patch sum) so DVE/PE patch work overlaps the conv phase.
  - r_pad and the conv pipeline run w-major so conv1x1 feeds w-blocked
    conv3x3 with contiguous ACT writes (innermost [1,128]).
  - ACT evicts PSUM sums to fp16 out tiles; output leaves the device as
    [s, chunk, h, (c, w32)]; pixel-shuffle + fp32 cast happen on the host.
"""

import sys

import numpy as np

sys.path.insert(0, "/opt/trn_rl_repo")

import ml_dtypes

import concourse.bass as bass
import concourse.mybir as mybir
import concourse.tile as tile
from concourse import bacc

F32 = mybir.dt.float32
# fp16: products are O(1-5); PSUM accumulation is fp32 so only the single
# rounding of each product matters (rel err ~1e-3, gate is 2e-2).
BF16 = mybir.dt.float16
BF_NP = np.float16

H = 128
W = 128
C = 64
M = 48  # compressed channels
S2 = 4  # scale_factor**2
K2 = 25  # k_up**2
SK = 100
HW = H * W
WPAD = W + 4  # w-padded pixel-major buffers
CW = C * WPAD  # 8448, free elems per (dy) plane
N_CORES = 8
NBLK = 32  # conv blocks of 4 w-columns x 128 h
BPC = 8  # conv blocks per chunk
WC = 32  # max chunk width (PSUM-limited: 64*32 fp32 = 4 banks)
CWC = C * WC  # 2048 free elems per patch-sum tile
SKZ = SK + S2  # F rows + stacked 1/Z rows, transposed together
FRC = SKZ * WC  # fr chunk tile pitch
RPF = 130 * 130
# (w0, width) patch chunks; narrow first chunk starts the DVE patch work
# ~50us earlier, narrow last chunk shortens the drain tail.
CHUNKS = [(0, 16), (16, 32), (48, 32), (80, 32), (112, 16)]


def _ap(t, extra_off, dims):
    """Raw AP on a tile handle `t` with free-offset `extra_off` (elements)
    and explicit [step, count] dims (dims[0] is the partition dim)."""
    base = t[:]
    return bass.AP(tensor=base.tensor, offset=base.offset + extra_off, ap=dims)


class _Pool:
    """Manually scoped tile pool."""

    def __init__(self, tc, **kw):
        self._cm = tc.tile_pool(**kw)
        self.pool = self._cm.__enter__()
        self._n = 0

    def tile(self, *a, tag=None, **kw):
        self._n += 1
        t = tag or f"t{self._n}"
        return self.pool.tile(*a, tag=t, name=t, **kw)

    def close(self):
        self._cm.__exit__(None, None, None)


def build_program():
    nc = bacc.Bacc("TRN2", target_bir_lowering=False, debug=False)

    xcw = nc.dram_tensor("xcw", [C + 1, HW], BF16, kind="ExternalInput")
    xt5d = nc.dram_tensor("xt5", [128, 5 * CW], BF16, kind="ExternalInput")
    w1t = nc.dram_tensor("w1t", [C + 1, M], BF16, kind="ExternalInput")
    wet = nc.dram_tensor("wet", [113, 6 * SK], BF16, kind="ExternalInput")
    sones = nc.dram_tensor("sones", [SK, S2], BF16, kind="ExternalInput")
    id128d = nc.dram_tensor("id128", [128, 128], BF16, kind="ExternalInput")
    onesr = nc.dram_tensor("onesr", [1, RPF], BF16, kind="ExternalInput")
    out = nc.dram_tensor("out", [S2, H * C * W], BF16, kind="ExternalOutput")

    with tile.TileContext(nc) as tc:
        cp = _Pool(tc, name="consts", bufs=1)
        w1t_sb = cp.tile([C + 1, M], BF16)
        nc.sync.dma_start(w1t_sb[:], w1t.ap())
        wet_sb = cp.tile([113, 6 * SK], BF16)
        nc.sync.dma_start(wet_sb[:], wet.ap())
        sones_sb = cp.tile([SK, S2], BF16)
        nc.sync.dma_start(sones_sb[:], sones.ap())
        id128_sb = cp.tile([128, 128], BF16)
        nc.sync.dma_start(id128_sb[:], id128d.ap())

        pp = _Pool(tc, name="persist", bufs=1)
        xt5 = pp.tile([128, 5 * CW], BF16)

        # ---- R_pad (w-major: free = wcol*130 + hrow) with 1-halo borders.
        # Two copies of R: A on partitions 0..48 at (wcol=1+w, hrow=1+h), B
        # on 64..112 at (wcol=w, hrow=1+h), so conv3x3 contracts taps
        # (ty,0)+(ty,1) in a single K=113 matmul ----
        p3 = _Pool(tc, name="rpad", bufs=1)
        r_pad = p3.tile([113, RPF], BF16)
        nc.gpsimd.memset(r_pad[:], 0.0)

        p4 = _Pool(tc, name="xb", bufs=2)
        psA = _Pool(tc, name="psA", bufs=1, space="PSUM")
        psB = _Pool(tc, name="psB", bufs=1, space="PSUM")
        psCD = _Pool(tc, name="psCD", bufs=1, space="PSUM")
        psF = _Pool(tc, name="psF", bufs=1, space="PSUM")
        psID = _Pool(tc, name="psID", bufs=1, space="PSUM")
        rzp = _Pool(tc, name="rz", bufs=2)
        fnp = _Pool(tc, name="fn", bufs=3)
        frp = _Pool(tc, name="fr", bufs=3)
        prp = _Pool(tc, name="pr", bufs=8)
        outp = _Pool(tc, name="outp", bufs=3)

        fn_tiles = {}  # conv block j -> f_norm tile [SK, (w4, h)]
        fr_tiles = {}  # chunk -> fr tile [128, (sk, w32)]
        xb_tiles = {}

        def load_xcw(cc):
            xb = p4.tile([C + 1, 512 * BPC], BF16, tag="xb")
            nc.sync.dma_start(
                xb[:], xcw.ap()[:, cc * 512 * BPC : (cc + 1) * 512 * BPC]
            )
            xb_tiles[cc] = xb

        def conv1x1(b):
            """Block b: 4 w-columns x 128 h, w-major pixels."""
            xb = xb_tiles[b // BPC]
            ji = b % BPC
            ps1 = psA.tile([M, 512], F32, tag="ps1")
            nc.tensor.matmul(
                ps1[:],
                w1t_sb[:],
                xb[:, ji * 512 : (ji + 1) * 512],
                start=True,
                stop=True,
            )
            nc.scalar.activation(
                _ap(r_pad, (1 + 4 * b) * 130 + 1, [[RPF, M], [130, 4], [1, H]]),
                ps1[:],
                mybir.ActivationFunctionType.Relu,
            )
            nc.scalar.activation(
                _ap(
                    r_pad,
                    64 * RPF + (4 * b) * 130 + 1,
                    [[RPF, M], [130, 4], [1, H]],
                ),
                ps1[:],
                mybir.ActivationFunctionType.Relu,
            )

        # conv3x3 slots: 3 paired (taps (ty,0)+(ty,1), K=113) + 3 single
        # (taps (ty,2), K=49)
        SLOTS = [(0, 113), (1, 113), (2, 113), (0, 49), (1, 49), (2, 49)]

        ft_tiles = {}
        psz_tiles = {}

        def conv3x3_a(j):
            """Block j: E -> exp F (rows 0..99) + Z row-sum matmul."""
            ps2 = psB.tile([SK, 512], F32, tag="ps2")
            for m, (ty, kk) in enumerate(SLOTS):
                off = ty + (4 * j + (0 if kk == 113 else 2)) * 130
                nc.tensor.matmul(
                    ps2[:],
                    wet_sb[0:kk, m * SK : (m + 1) * SK],
                    _ap(r_pad, off, [[RPF, kk], [130, 4], [1, H]]),
                    start=(m == 0),
                    stop=(m == len(SLOTS) - 1),
                )
            ft = fnp.tile([SK, 512], BF16, tag="fn")
            nc.scalar.activation(ft[:], ps2[:], mybir.ActivationFunctionType.Exp)
            psz = psCD.tile([S2, 512], F32, tag="psz")
            nc.tensor.matmul(psz[:], sones_sb[:], ft[:], start=True, stop=True)
            ft_tiles[j] = ft
            psz_tiles[j] = psz

        def conv3x3_b(j):
            """Block j: 1/Z on the Activation engine.  Grouped per block
            pair after both exps so the exp<->reciprocal activation-table
            reload happens once per pair, not per block.  (bass's wrapper
            blocks Reciprocal for accuracy; softmax denominators are
            O(10-100) and the 2e-2 gate tolerates the table's error.)"""
            psz = psz_tiles.pop(j)
            rzt = rzp.tile([S2, 512], BF16, tag="rz16")
            eng = nc.scalar
            ins_ = [eng.lower_ap(psz[:])] + [
                mybir.ImmediateValue(dtype=mybir.dt.float32, value=v)
                for v in (0.0, 1.0, 0.0)
            ]
            eng.add_instruction(
                mybir.InstActivation(
                    name=eng.bass.get_next_instruction_name(),
                    func=mybir.ActivationFunctionType.Reciprocal,
                    ins=ins_,
                    outs=[eng.lower_ap(rzt[:])],
                )
            )
            fn_tiles[j] = (ft_tiles.pop(j), rzt)

        def chunk_of_block(b):
            for kc, (w0, wd) in enumerate(CHUNKS):
                if w0 <= 4 * b < w0 + wd:
                    return kc
            raise AssertionError(b)

        def transpose_pair(j0):
            """Transpose finished blocks j0, j0+1 (8 columns of F + 1/Z
            rows) into the fr chunk tile fr_c[h, skz*wd + w_local]."""
            kc = chunk_of_block(j0)
            w0, wd = CHUNKS[kc]
            if kc not in fr_tiles:
                fr_tiles[kc] = frp.tile([128, FRC], BF16, tag="frc")
            frc = fr_tiles[kc]
            wl = 4 * j0 - w0
            pst = psF.tile([128, 8 * SKZ], BF16, tag="pst")
            for wi in range(8):
                ft, rzt = fn_tiles[j0 + wi // 4]
                nc.tensor.transpose(
                    pst[:, wi * SKZ : wi * SKZ + SK],
                    _ap(ft, (wi % 4) * H, [[512, SK], [1, H]]),
                    id128_sb[0:SK, 0:SK],
                )
                nc.tensor.transpose(
                    pst[:, wi * SKZ + SK : (wi + 1) * SKZ],
                    _ap(rzt, (wi % 4) * H, [[512, S2], [1, H]]),
                    id128_sb[0:S2, 0:S2],
                )
            nc.scalar.copy(
                _ap(frc, wl, [[FRC, 128], [1, 8], [wd, SKZ]]),
                _ap(pst, 0, [[8 * SKZ, 128], [SKZ, 8], [1, SKZ]]),
            )
            del fn_tiles[j0], fn_tiles[j0 + 1]
            if 4 * (j0 + 2) == w0 + wd:
                # chunk complete: normalize F in place, one broadcast mul:
                # fr[h, (s,k,w)] *= fr[h, 100+s, w]
                nc.vector.tensor_mul(
                    _ap(frc, 0, [[FRC, 128], [K2 * wd, S2], [wd, K2], [1, wd]]),
                    _ap(frc, 0, [[FRC, 128], [K2 * wd, S2], [wd, K2], [1, wd]]),
                    _ap(frc, SK * wd, [[FRC, 128], [wd, S2], [0, K2], [1, wd]]),
                )
                return kc
            return None

        def patch_s(kc, s):
            """Patch sum for chunk kc (w-columns [w0, w0+wd)), one s: DVE
            makes 25 product planes, PE identity-accumulates them in PSUM,
            ACT evicts to fp16, DMA to DRAM."""
            w0, wd = CHUNKS[kc]
            cwd = C * wd
            frc = fr_tiles[kc]
            psid = psID.tile([128, CWC], F32, tag="psid")
            for k in range(K2):
                dy, dx = k // 5, k % 5
                pk = prp.tile([128, CWC], BF16, tag="pk")
                nc.vector.tensor_mul(
                    _ap(pk, 0, [[CWC, 128], [wd, C], [1, wd]]),
                    _ap(
                        xt5,
                        dy * CW + dx + w0,
                        [[5 * CW, 128], [WPAD, C], [1, wd]],
                    ),
                    _ap(frc, (s * K2 + k) * wd, [[FRC, 128], [0, C], [1, wd]]),
                )
                for q in range(cwd // 512):
                    nc.tensor.matmul(
                        psid[:, q * 512 : (q + 1) * 512],
                        id128_sb[:],
                        pk[:, q * 512 : (q + 1) * 512],
                        start=(k == 0),
                        stop=(k == K2 - 1),
                        skip_group_check=True,
                    )
            ot = outp.tile([128, CWC], BF16, tag="ot")
            nc.scalar.copy(ot[:, 0:cwd], psid[:, 0:cwd])
            nc.sync.dma_start(
                bass.AP(
                    tensor=out,
                    offset=s * H * C * W + w0 * H * C,
                    ap=[[cwd, 128], [1, cwd]],
                ),
                ot[:, 0:cwd],
            )

        # ---- pipelined issue: per chunk iteration, conv1x1 runs one chunk
        # ahead of conv3x3; patch sum trails conv3x3 by one chunk.  Work is
        # interleaved at block-pair granularity so PE alternates between
        # conv matmuls and identity-sum matmuls and the product ring stays
        # shallow. ----
        # DMA order matters: the 16 DMA engines drain queues in issue order,
        # so the first conv input chunk goes first, then the big xt5 load
        # (needed ~30us in), then r_pad's ones rows.
        load_xcw(0)
        nc.sync.dma_start(xt5[:], xt5d.ap())
        nc.sync.dma_start(_ap(r_pad, M * RPF, [[RPF, 1], [1, RPF]]), onesr.ap())
        nc.sync.dma_start(
            _ap(r_pad, (M + 64) * RPF, [[RPF, 1], [1, RPF]]), onesr.ap()
        )
        # PE warmup: keep the array busy through the DMA wait so the
        # p-state governor ramps the clock before the real conv starts.
        pswarm = psID.tile([128, CWC], F32, tag="psid")
        for i in range(8):
            nc.tensor.matmul(
                pswarm[:, 0:128], id128_sb[:], id128_sb[:, 0:128],
                start=True, stop=True,
            )

        # Pair-granular pipeline: conv1x1 leads conv3x3 by one block pair;
        # each finished chunk queues its 4 patch_s slices, drained one per
        # iteration so patch work interleaves with the next chunk's conv.
        NPAIR = NBLK // 2
        backlog = []
        for p in range(NPAIR + 1):
            if p < NPAIR:
                if p % 4 == 0 and p // 4 + 1 < NBLK // BPC:
                    load_xcw(p // 4 + 1)
                conv1x1(2 * p)
                conv1x1(2 * p + 1)
            # In steady state, issue the patch slice BEFORE the conv pair
            # so PE consumes DVE's product ring immediately and runs conv
            # afterwards while the ring refills (keeps DVE from stalling on
            # ring WAR during conv bursts).  Keep one slice in reserve so
            # DVE doesn't run dry at chunk boundaries.
            early = p >= 3
            if early and (len(backlog) > 1 or (backlog and p >= NPAIR)):
                patch_s(*backlog.pop(0))
            if 1 <= p:
                j = 2 * (p - 1)
                conv3x3_a(j)
                conv3x3_a(j + 1)
                conv3x3_b(j)
                conv3x3_b(j + 1)
                done = transpose_pair(j)
                if done is not None:
                    backlog.extend((done, s) for s in range(S2))
            if not early and (len(backlog) > 1 or (backlog and p >= NPAIR)):
                patch_s(*backlog.pop(0))
        while backlog:
            patch_s(*backlog.pop(0))

        outp.close()
        prp.close()
        frp.close()
        fnp.close()
        rzp.close()
        psID.close()
        psF.close()
        psCD.close()
        psB.close()
        psA.close()
        p4.close()
        p3.close()
        pp.close()
        cp.close()
    nc.compile()
    return nc


def host_inputs(x_img, w_compress, b_compress, w_encoder, b_encoder):
    """Per-core input map for one image [C, H, W] (all fp16)."""
    x_img = np.asarray(x_img, np.float32)
    # w-major pixels for conv1x1: xcw[c, w*H + h] = x[c, h, w]
    xcw = np.concatenate(
        [
            np.ascontiguousarray(x_img.transpose(0, 2, 1)).reshape(C, HW),
            np.ones((1, HW), np.float32),
        ],
        axis=0,
    ).astype(BF_NP)
    # pixel-major, w-padded, 5 dy-shifted planes: xt5[h, dy, c, wp]
    #   = xpad[c, h + dy, wp]  (xpad has pad 2 on h and w)
    xpad = np.pad(x_img, ((0, 0), (2, 2), (2, 2))).astype(BF_NP)
    xt5 = np.stack([xpad[:, dy : dy + H, :] for dy in range(5)], axis=0)
    xt5 = np.ascontiguousarray(xt5.transpose(2, 0, 1, 3)).reshape(128, 5 * CW)
    w1t = np.concatenate(
        [w_compress[:, :, 0, 0].T, b_compress[None, :]], axis=0
    ).astype(BF_NP)
    # paired layout: slots 0-2 = taps (ty,0) on rows 0..47 + (ty,1) on rows
    # 64..111; slots 3-5 = single taps (ty,2).  Bias rides the all-ones rows
    # (48 for A, 112 for B) on the center tap (1,1) = slot 1's B half.
    wetm = np.zeros((113, 6, SK), np.float32)
    for ty in range(3):
        wetm[:M, ty, :] = w_encoder[:, :, ty, 0].T
        wetm[64 : 64 + M, ty, :] = w_encoder[:, :, ty, 1].T
        wetm[:M, 3 + ty, :] = w_encoder[:, :, ty, 2].T
    wetm[112, 1, :] = b_encoder
    son = np.zeros((SK, S2), np.float32)
    for s in range(S2):
        son[s * K2 : (s + 1) * K2, s] = 1.0
    return {
        "xcw": xcw,
        "xt5": xt5,
        "w1t": w1t,
        "wet": wetm.reshape(113, 6 * SK).astype(BF_NP),
        "sones": son.astype(BF_NP),
        "id128": np.eye(128, dtype=BF_NP),
        "onesr": np.ones((1, RPF), BF_NP),
    }


def _unshuffle(dev_out):
    """[S2, H*C*W] chunk-major fp16 -> [64, 256, 256] fp32 pixel-shuffled."""
    a = np.asarray(dev_out).reshape(S2, H * C * W)
    full = np.empty((S2, H, 16, 2, 2, W), np.float16)
    for w0, wd in CHUNKS:
        seg = a[:, w0 * H * C : (w0 + wd) * H * C].reshape(S2, H, 16, 2, 2, wd)
        full[..., w0 : w0 + wd] = seg
    # (s, h, c4, c2, c1, w) -> (s, c4, h, c2, w, c1)
    full = full.transpose(0, 2, 1, 3, 5, 4)
    return np.ascontiguousarray(full).reshape(C, 2 * H, 2 * W).astype(np.float32)


_CACHE = {}


def kernel(x, w_compress, b_compress, w_encoder, b_encoder):
    x = np.asarray(x, np.float32)
    if "nc" not in _CACHE:
        _CACHE["nc"] = build_program()
    nc = _CACHE["nc"]
    in_maps = [
        host_inputs(
            x[i],
            np.asarray(w_compress, np.float32),
            np.asarray(b_compress, np.float32),
            np.asarray(w_encoder, np.float32),
            np.asarray(b_encoder, np.float32),
        )
        for i in range(N_CORES)
    ]
    from concourse.bass_utils import run_bass_kernel_spmd

    res = run_bass_kernel_spmd(nc, in_maps, core_ids=list(range(N_CORES)))
    return np.stack(
        [_unshuffle(res.results[i]["out"]) for i in range(N_CORES)], axis=0
    )


# revision 30
# speedup vs baseline: 1.1674x; 1.1674x over previous
"""CARAFE kernel for Trainium2 (8 NeuronCores, batch-parallel), v4.

Reference computation per image:
  R = relu(conv1x1(x, w_compress, b_compress))          [48, 128, 128]
  E = conv3x3(R, w_encoder, b_encoder, pad=1)           [100, 128, 128]
  Y = softmax over k of E.reshape(4, 25, H, W)          (s, k, h, w)
  out[s,c,h,w] = sum_k Y[s,k,h,w] * xpad[c, h+dy, w+dx] (k=(dy,dx), 5x5, pad 2)
  pixel-shuffle: out_ref[s*16 + c//4, 2h + (c//2)%2, 2w + c%2] = out[s,c,h,w]

v4 design (vs v2/v3):
  - The 25-tap patch sum is elementwise (per-pixel weights), but only the
    PRODUCTS need an elementwise engine.  The k-SUM moves to the Tensor
    engine: DVE writes each product plane P_k = Y_k * x_k to SBUF, and PE
    accumulates all 25 planes into PSUM via identity matmuls
    (psum[h,n] += sum_p I[p,h] P_k[p,n]).  This halves DVE work - DVE does
    25 muls per (s, chunk) instead of 25 muls + 24 adds - and the
    accumulation becomes exact fp32.
  - gpsimd does NO elementwise work: measured on HW, concurrent Pool
    activity slows DVE ops ~4x (global SBUF port contention), making any
    gpsimd offload net-negative.
  - Whole dataflow is pipelined in 32-column chunks (conv1x1 -> conv3x3 ->
    softmax -> patch sum) so DVE/PE patch work overlaps the conv phase.
  - r_pad and the conv pipeline run w-major so conv1x1 feeds w-blocked
    conv3x3 with contiguous ACT writes (innermost [1,128]).
  - ACT evicts PSUM sums to fp16 out tiles; output leaves the device as
    [s, chunk, h, (c, w32)]; pixel-shuffle + fp32 cast happen on the host.
"""

import sys

import numpy as np

sys.path.insert(0, "/opt/trn_rl_repo")

import ml_dtypes

import concourse.bass as bass
import concourse.mybir as mybir
import concourse.tile as tile
from concourse import bacc

F32 = mybir.dt.float32
# fp16: products are O(1-5); PSUM accumulation is fp32 so only the single
# rounding of each product matters (rel err ~1e-3, gate is 2e-2).
BF16 = mybir.dt.float16
BF_NP = np.float16

H = 128
W = 128
C = 64
M = 48  # compressed channels
S2 = 4  # scale_factor**2
K2 = 25  # k_up**2
SK = 100
HW = H * W
WPAD = W + 4  # w-padded pixel-major buffers
CW = C * WPAD  # 8448, free elems per (dy) plane
N_CORES = 8
NBLK = 32  # conv blocks of 4 w-columns x 128 h
BPC = 8  # conv blocks per chunk
WC = 32  # max chunk width (PSUM-limited: 64*32 fp32 = 4 banks)
CWC = C * WC  # 2048 free elems per patch-sum tile
SKZ = SK + S2  # F rows + stacked 1/Z rows, transposed together
FRC = SKZ * WC  # fr chunk tile pitch
RPF = 130 * 130
# (w0, width) patch chunks; narrow first chunk starts the DVE patch work
# ~50us earlier, narrow last chunk shortens the drain tail.
CHUNKS = [(0, 16), (16, 32), (48, 32), (80, 32), (112, 16)]


def _ap(t, extra_off, dims):
    """Raw AP on a tile handle `t` with free-offset `extra_off` (elements)
    and explicit [step, count] dims (dims[0] is the partition dim)."""
    base = t[:]
    return bass.AP(tensor=base.tensor, offset=base.offset + extra_off, ap=dims)


class _Pool:
    """Manually scoped tile pool."""

    def __init__(self, tc, **kw):
        self._cm = tc.tile_pool(**kw)
        self.pool = self._cm.__enter__()
        self._n = 0

    def tile(self, *a, tag=None, **kw):
        self._n += 1
        t = tag or f"t{self._n}"
        return self.pool.tile(*a, tag=t, name=t, **kw)

    def close(self):
        self._cm.__exit__(None, None, None)


def build_program():
    nc = bacc.Bacc("TRN2", target_bir_lowering=False, debug=False)

    xcw = nc.dram_tensor("xcw", [C + 1, HW], BF16, kind="ExternalInput")
    xt5d = nc.dram_tensor("xt5", [128, 5 * CW], BF16, kind="ExternalInput")
    w1t = nc.dram_tensor("w1t", [C + 1, M], BF16, kind="ExternalInput")
    wet = nc.dram_tensor("wet", [113, 6 * SK], BF16, kind="ExternalInput")
    sones = nc.dram_tensor("sones", [SK, S2], BF16, kind="ExternalInput")
    id128d = nc.dram_tensor("id128", [128, 128], BF16, kind="ExternalInput")
    onesr = nc.dram_tensor("onesr", [1, RPF], BF16, kind="ExternalInput")
    out = nc.dram_tensor("out", [S2, H * C * W], BF16, kind="ExternalOutput")

    with tile.TileContext(nc) as tc:
        cp = _Pool(tc, name="consts", bufs=1)
        w1t_sb = cp.tile([C + 1, M], BF16)
        nc.sync.dma_start(w1t_sb[:], w1t.ap())
        wet_sb = cp.tile([113, 6 * SK], BF16)
        nc.sync.dma_start(wet_sb[:], wet.ap())
        sones_sb = cp.tile([SK, S2], BF16)
        nc.sync.dma_start(sones_sb[:], sones.ap())
        id128_sb = cp.tile([128, 128], BF16)
        nc.sync.dma_start(id128_sb[:], id128d.ap())

        pp = _Pool(tc, name="persist", bufs=1)
        xt5 = pp.tile([128, 5 * CW], BF16)

        # ---- R_pad (w-major: free = wcol*130 + hrow) with 1-halo borders.
        # Two copies of R: A on partitions 0..48 at (wcol=1+w, hrow=1+h), B
        # on 64..112 at (wcol=w, hrow=1+h), so conv3x3 contracts taps
        # (ty,0)+(ty,1) in a single K=113 matmul ----
        p3 = _Pool(tc, name="rpad", bufs=1)
        r_pad = p3.tile([113, RPF], BF16)
        nc.gpsimd.memset(r_pad[:], 0.0)

        p4 = _Pool(tc, name="xb", bufs=2)
        psA = _Pool(tc, name="psA", bufs=1, space="PSUM")
        psB = _Pool(tc, name="psB", bufs=1, space="PSUM")
        psCD = _Pool(tc, name="psCD", bufs=1, space="PSUM")
        psF = _Pool(tc, name="psF", bufs=1, space="PSUM")
        psID = _Pool(tc, name="psID", bufs=1, space="PSUM")
        rzp = _Pool(tc, name="rz", bufs=2)
        fnp = _Pool(tc, name="fn", bufs=3)
        frp = _Pool(tc, name="fr", bufs=3)
        prp = _Pool(tc, name="pr", bufs=8)
        outp = _Pool(tc, name="outp", bufs=3)

        fn_tiles = {}  # conv block j -> f_norm tile [SK, (w4, h)]
        fr_tiles = {}  # chunk -> fr tile [128, (sk, w32)]
        xb_tiles = {}

        def load_xcw(cc):
            xb = p4.tile([C + 1, 512 * BPC], BF16, tag="xb")
            nc.sync.dma_start(
                xb[:], xcw.ap()[:, cc * 512 * BPC : (cc + 1) * 512 * BPC]
            )
            xb_tiles[cc] = xb

        def conv1x1(b):
            """Block b: 4 w-columns x 128 h, w-major pixels."""
            xb = xb_tiles[b // BPC]
            ji = b % BPC
            ps1 = psA.tile([M, 512], F32, tag="ps1")
            nc.tensor.matmul(
                ps1[:],
                w1t_sb[:],
                xb[:, ji * 512 : (ji + 1) * 512],
                start=True,
                stop=True,
            )
            nc.scalar.activation(
                _ap(r_pad, (1 + 4 * b) * 130 + 1, [[RPF, M], [130, 4], [1, H]]),
                ps1[:],
                mybir.ActivationFunctionType.Relu,
            )
            nc.scalar.activation(
                _ap(
                    r_pad,
                    64 * RPF + (4 * b) * 130 + 1,
                    [[RPF, M], [130, 4], [1, H]],
                ),
                ps1[:],
                mybir.ActivationFunctionType.Relu,
            )

        # conv3x3 slots: 3 paired (taps (ty,0)+(ty,1), K=113) + 3 single
        # (taps (ty,2), K=49)
        SLOTS = [(0, 113), (1, 113), (2, 113), (0, 49), (1, 49), (2, 49)]

        ft_tiles = {}
        psz_tiles = {}

        def conv3x3_a(j):
            """Block j: E -> exp F (rows 0..99) + Z row-sum matmul.
            Chunk0's four blocks quad-buffer in the (ramp-idle) identity-sum
            PSUM tile so their matmul groups never wait on exp reads."""
            if j < 4:
                ps2 = pswarm[0:SK, (j % 4) * 512 : (j % 4 + 1) * 512]
            else:
                ps2 = psB.tile([SK, 512], F32, tag="ps2")[:]
            for m, (ty, kk) in enumerate(SLOTS):
                off = ty + (4 * j + (0 if kk == 113 else 2)) * 130
                nc.tensor.matmul(
                    ps2,
                    wet_sb[0:kk, m * SK : (m + 1) * SK],
                    _ap(r_pad, off, [[RPF, kk], [130, 4], [1, H]]),
                    start=(m == 0),
                    stop=(m == len(SLOTS) - 1),
                )
            ft = fnp.tile([SK, 512], BF16, tag="fn")
            nc.scalar.activation(ft[:], ps2, mybir.ActivationFunctionType.Exp)
            psz = psCD.tile([S2, 512], F32, tag="psz")
            nc.tensor.matmul(psz[:], sones_sb[:], ft[:], start=True, stop=True)
            ft_tiles[j] = ft
            psz_tiles[j] = psz

        def conv3x3_b(j):
            """Block j: 1/Z on the Activation engine.  Grouped per block
            pair after both exps so the exp<->reciprocal activation-table
            reload happens once per pair, not per block.  (bass's wrapper
            blocks Reciprocal for accuracy; softmax denominators are
            O(10-100) and the 2e-2 gate tolerates the table's error.)"""
            psz = psz_tiles.pop(j)
            rzt = rzp.tile([S2, 512], BF16, tag="rz16")
            eng = nc.scalar
            ins_ = [eng.lower_ap(psz[:])] + [
                mybir.ImmediateValue(dtype=mybir.dt.float32, value=v)
                for v in (0.0, 1.0, 0.0)
            ]
            eng.add_instruction(
                mybir.InstActivation(
                    name=eng.bass.get_next_instruction_name(),
                    func=mybir.ActivationFunctionType.Reciprocal,
                    ins=ins_,
                    outs=[eng.lower_ap(rzt[:])],
                )
            )
            fn_tiles[j] = (ft_tiles.pop(j), rzt)

        def chunk_of_block(b):
            for kc, (w0, wd) in enumerate(CHUNKS):
                if w0 <= 4 * b < w0 + wd:
                    return kc
            raise AssertionError(b)

        def transpose_pair(j0):
            """Transpose finished blocks j0, j0+1 (8 columns of F + 1/Z
            rows) into the fr chunk tile fr_c[h, skz*wd + w_local]."""
            kc = chunk_of_block(j0)
            w0, wd = CHUNKS[kc]
            if kc not in fr_tiles:
                fr_tiles[kc] = frp.tile([128, FRC], BF16, tag="frc")
            frc = fr_tiles[kc]
            wl = 4 * j0 - w0
            pst = psF.tile([128, 8 * SKZ], BF16, tag="pst")
            for wi in range(8):
                ft, rzt = fn_tiles[j0 + wi // 4]
                nc.tensor.transpose(
                    pst[:, wi * SKZ : wi * SKZ + SK],
                    _ap(ft, (wi % 4) * H, [[512, SK], [1, H]]),
                    id128_sb[0:SK, 0:SK],
                )
                nc.tensor.transpose(
                    pst[:, wi * SKZ + SK : (wi + 1) * SKZ],
                    _ap(rzt, (wi % 4) * H, [[512, S2], [1, H]]),
                    id128_sb[0:S2, 0:S2],
                )
            nc.scalar.copy(
                _ap(frc, wl, [[FRC, 128], [1, 8], [wd, SKZ]]),
                _ap(pst, 0, [[8 * SKZ, 128], [SKZ, 8], [1, SKZ]]),
            )
            del fn_tiles[j0], fn_tiles[j0 + 1]
            if 4 * (j0 + 2) == w0 + wd:
                # chunk complete: normalize F in place, one broadcast mul:
                # fr[h, (s,k,w)] *= fr[h, 100+s, w]
                nc.vector.tensor_mul(
                    _ap(frc, 0, [[FRC, 128], [K2 * wd, S2], [wd, K2], [1, wd]]),
                    _ap(frc, 0, [[FRC, 128], [K2 * wd, S2], [wd, K2], [1, wd]]),
                    _ap(frc, SK * wd, [[FRC, 128], [wd, S2], [0, K2], [1, wd]]),
                )
                return kc
            return None

        def patch_s(kc, s):
            """Patch sum for chunk kc (w-columns [w0, w0+wd)), one s: DVE
            makes 25 product planes, PE identity-accumulates them in PSUM,
            ACT evicts to fp16, DMA to DRAM."""
            w0, wd = CHUNKS[kc]
            cwd = C * wd
            frc = fr_tiles[kc]
            psid = psID.tile([128, CWC], F32, tag="psid")
            for k in range(K2):
                dy, dx = k // 5, k % 5
                pk = prp.tile([128, CWC], BF16, tag="pk")
                nc.vector.tensor_mul(
                    _ap(pk, 0, [[CWC, 128], [wd, C], [1, wd]]),
                    _ap(
                        xt5,
                        dy * CW + dx + w0,
                        [[5 * CW, 128], [WPAD, C], [1, wd]],
                    ),
                    _ap(frc, (s * K2 + k) * wd, [[FRC, 128], [0, C], [1, wd]]),
                )
                for q in range(cwd // 512):
                    nc.tensor.matmul(
                        psid[:, q * 512 : (q + 1) * 512],
                        id128_sb[:],
                        pk[:, q * 512 : (q + 1) * 512],
                        start=(k == 0),
                        stop=(k == K2 - 1),
                        skip_group_check=True,
                    )
            ot = outp.tile([128, CWC], BF16, tag="ot")
            nc.scalar.copy(ot[:, 0:cwd], psid[:, 0:cwd])
            nc.sync.dma_start(
                bass.AP(
                    tensor=out,
                    offset=s * H * C * W + w0 * H * C,
                    ap=[[cwd, 128], [1, cwd]],
                ),
                ot[:, 0:cwd],
            )

        # ---- pipelined issue: per chunk iteration, conv1x1 runs one chunk
        # ahead of conv3x3; patch sum trails conv3x3 by one chunk.  Work is
        # interleaved at block-pair granularity so PE alternates between
        # conv matmuls and identity-sum matmuls and the product ring stays
        # shallow. ----
        # DMA order matters: the 16 DMA engines drain queues in issue order,
        # so the first conv input chunk goes first, then the big xt5 load
        # (needed ~30us in), then r_pad's ones rows.
        load_xcw(0)
        nc.sync.dma_start(xt5[:], xt5d.ap())
        nc.sync.dma_start(_ap(r_pad, M * RPF, [[RPF, 1], [1, RPF]]), onesr.ap())
        nc.sync.dma_start(
            _ap(r_pad, (M + 64) * RPF, [[RPF, 1], [1, RPF]]), onesr.ap()
        )
        # PE warmup: keep the array busy through the DMA wait so the
        # p-state governor ramps the clock before the real conv starts.
        pswarm = psID.tile([128, CWC], F32, tag="psid")
        for i in range(8):
            nc.tensor.matmul(
                pswarm[:, 0:128], id128_sb[:], id128_sb[:, 0:128],
                start=True, stop=True,
            )

        # Pair-granular pipeline: conv1x1 leads conv3x3 by one block pair;
        # each finished chunk queues its 4 patch_s slices, drained one per
        # iteration so patch work interleaves with the next chunk's conv.
        NPAIR = NBLK // 2
        backlog = []
        for p in range(NPAIR + 1):
            if p < NPAIR:
                if p % 4 == 0 and p // 4 + 1 < NBLK // BPC:
                    load_xcw(p // 4 + 1)
                conv1x1(2 * p)
                conv1x1(2 * p + 1)
            if 1 <= p:
                j = 2 * (p - 1)
                conv3x3_a(j)
                conv3x3_a(j + 1)
                conv3x3_b(j)
                conv3x3_b(j + 1)
                done = transpose_pair(j)
                if done is not None:
                    backlog.extend((done, s) for s in range(S2))
            # keep one slice in reserve so DVE doesn't run dry at chunk
            # boundaries while the next chunk's fr is still in flight
            if len(backlog) > 1 or (backlog and p >= NPAIR):
                patch_s(*backlog.pop(0))
        while backlog:
            patch_s(*backlog.pop(0))

        outp.close()
        prp.close()
        frp.close()
        fnp.close()
        rzp.close()
        psID.close()
        psF.close()
        psCD.close()
        psB.close()
        psA.close()
        p4.close()
        p3.close()
        pp.close()
        cp.close()
    nc.compile()
    return nc


def host_inputs(x_img, w_compress, b_compress, w_encoder, b_encoder):
    """Per-core input map for one image [C, H, W] (all fp16)."""
    x_img = np.asarray(x_img, np.float32)
    # w-major pixels for conv1x1: xcw[c, w*H + h] = x[c, h, w]
    xcw = np.concatenate(
        [
            np.ascontiguousarray(x_img.transpose(0, 2, 1)).reshape(C, HW),
            np.ones((1, HW), np.float32),
        ],
        axis=0,
    ).astype(BF_NP)
    # pixel-major, w-padded, 5 dy-shifted planes: xt5[h, dy, c, wp]
    #   = xpad[c, h + dy, wp]  (xpad has pad 2 on h and w)
    xpad = np.pad(x_img, ((0, 0), (2, 2), (2, 2))).astype(BF_NP)
    xt5 = np.stack([xpad[:, dy : dy + H, :] for dy in range(5)], axis=0)
    xt5 = np.ascontiguousarray(xt5.transpose(2, 0, 1, 3)).reshape(128, 5 * CW)
    w1t = np.concatenate(
        [w_compress[:, :, 0, 0].T, b_compress[None, :]], axis=0
    ).astype(BF_NP)
    # paired layout: slots 0-2 = taps (ty,0) on rows 0..47 + (ty,1) on rows
    # 64..111; slots 3-5 = single taps (ty,2).  Bias rides the all-ones rows
    # (48 for A, 112 for B) on the center tap (1,1) = slot 1's B half.
    wetm = np.zeros((113, 6, SK), np.float32)
    for ty in range(3):
        wetm[:M, ty, :] = w_encoder[:, :, ty, 0].T
        wetm[64 : 64 + M, ty, :] = w_encoder[:, :, ty, 1].T
        wetm[:M, 3 + ty, :] = w_encoder[:, :, ty, 2].T
    wetm[112, 1, :] = b_encoder
    son = np.zeros((SK, S2), np.float32)
    for s in range(S2):
        son[s * K2 : (s + 1) * K2, s] = 1.0
    return {
        "xcw": xcw,
        "xt5": xt5,
        "w1t": w1t,
        "wet": wetm.reshape(113, 6 * SK).astype(BF_NP),
        "sones": son.astype(BF_NP),
        "id128": np.eye(128, dtype=BF_NP),
        "onesr": np.ones((1, RPF), BF_NP),
    }


def _unshuffle(dev_out):
    """[S2, H*C*W] chunk-major fp16 -> [64, 256, 256] fp32 pixel-shuffled."""
    a = np.asarray(dev_out).reshape(S2, H * C * W)
    full = np.empty((S2, H, 16, 2, 2, W), np.float16)
    for w0, wd in CHUNKS:
        seg = a[:, w0 * H * C : (w0 + wd) * H * C].reshape(S2, H, 16, 2, 2, wd)
        full[..., w0 : w0 + wd] = seg
    # (s, h, c4, c2, c1, w) -> (s, c4, h, c2, w, c1)
    full = full.transpose(0, 2, 1, 3, 5, 4)
    return np.ascontiguousarray(full).reshape(C, 2 * H, 2 * W).astype(np.float32)


_CACHE = {}


def kernel(x, w_compress, b_compress, w_encoder, b_encoder):
    x = np.asarray(x, np.float32)
    if "nc" not in _CACHE:
        _CACHE["nc"] = build_program()
    nc = _CACHE["nc"]
    in_maps = [
        host_inputs(
            x[i],
            np.asarray(w_compress, np.float32),
            np.asarray(b_compress, np.float32),
            np.asarray(w_encoder, np.float32),
            np.asarray(b_encoder, np.float32),
        )
        for i in range(N_CORES)
    ]
    from concourse.bass_utils import run_bass_kernel_spmd

    res = run_bass_kernel_spmd(nc, in_maps, core_ids=list(range(N_CORES)))
    return np.stack(
        [_unshuffle(res.results[i]["out"]) for i in range(N_CORES)], axis=0
    )
